# revision 1
# baseline (speedup 1.0000x reference)
"""Trainium2 Bass kernel for the dense-MLP Bayesian log-joint problem.

Computes, for fixed MLP weights:
    h1 = relu(X @ W1.T + b1); h2 = relu(h1 @ W2.T + b2)
    logits = h2 @ W3.T + b3
    out = sum_i log_softmax(logits)[i, Y[i]] + log MVN(0, 100 I)(params)

Strategy: data-parallel over 8 NeuronCores. Each core gets 2048 rows of
X/Y plus a replicated copy of the (small) weights, computes its partial
log-likelihood sum on-device, and the host adds the partials plus the
closed-form Gaussian prior term.

On-device layout is "transposed activations": every matmul keeps the
contraction dim on SBUF partitions. The host pre-transposes X and the
weight matrices into PE-friendly tiles so no on-device transposes are
needed.

Matmuls run in fp8 (e4m3) with DoubleRow perf mode: inputs are scaled by
powers of two into fp8 range on the host, and the PSUM results are
rescaled exactly inside the (fp32) activation that applies bias+relu.
fp32 PSUM accumulation throughout; the log-softmax epilogue is fp32.
The final scalar is dominated by the prior constant d*log(2*pi*100), so
the quantized forward error lands at ~1e-7 relative (measured 5.7e-8
against an f64 reference on the real inputs; vs the f32 jax reference
both fp8 and bf16 modes measure 0.0 relative error).

Measured on 8 axon TRN2 cores (hardware For_i loop, paired trip-count
differencing, same-process A/B): ~208 us per full evaluation with the
half-batch-split layer-3 PSUM (vs ~232 us unsplit in the same process),
~1000 TFLOP/s aggregate, ~78% of theoretical fp8 peak; bf16 mode ~473 us.
"""

import math

import numpy as np
import ml_dtypes

N = 16384
D = 1024
H = 2048
C = 10
CP = 16  # classes padded to 16 so layer-3 DoubleRow satisfies step%16==0
N_CORES = 8
NL = N // N_CORES  # 2048 rows per core
PRIOR_VAR = 100.0

BF16 = ml_dtypes.bfloat16
E4M3 = ml_dtypes.float8_e4m3  # TRN fp8e4: max normal +-240

# Power-of-two scales that place X / weights / hidden activations into
# fp8e4m3's sweet spot. All rescales are exact in fp32.
SX = 16.0
SW = 128.0
SH = 16.0

_compiled = {}


def _emit(tc, ctx, aps, repeat, stage="full", hw_loop=False, prec="fp8",
          wbufs=3):
    import contextlib

    import concourse.bass as bass
    from concourse import mybir

    nc = tc.nc
    f32 = mybir.dt.float32
    AF = mybir.ActivationFunctionType
    fp8 = prec == "fp8"
    dt_in = mybir.dt.float8e4 if fp8 else mybir.dt.bfloat16
    perf_mode = mybir.MatmulPerfMode.DoubleRow if fp8 else None
    kstep = 2 if fp8 else 1
    # PSUM -> activation rescales (exact powers of two)
    s12 = SH / (SX * SW) if fp8 else 1.0   # layer1 out scale; layer2 identical
    s2 = SH / (SH * SW) if fp8 else 1.0
    s3 = 1.0 / (SH * SW) if fp8 else 1.0

    xt, w1, w2, w3, b1, b2, b3, oh, out = (
        aps["xt"], aps["w1"], aps["w2"], aps["w3"],
        aps["b1"], aps["b2"], aps["b3"], aps["oh"], aps["out"],
    )

    KD = D // 128   # 8  k-tiles for layer 1
    KH = H // 128   # 16 k-tiles for layers 2/3, and m-tiles for layers 1/2
    NS = NL // 512  # 4  n-slices of the batch free dim

    consts = ctx.enter_context(tc.tile_pool(name="consts", bufs=1))
    acts = ctx.enter_context(tc.tile_pool(name="acts", bufs=1))
    w1p = ctx.enter_context(tc.tile_pool(name="w1p", bufs=wbufs))
    w2p = ctx.enter_context(tc.tile_pool(name="w2p", bufs=wbufs))
    psum = ctx.enter_context(tc.tile_pool(name="psum", bufs=2, space="PSUM"))
    epil = ctx.enter_context(tc.tile_pool(name="epil", bufs=2))

    # Constants / resident tensors
    xt_sb = consts.tile([128, KD, NL], dt_in, name="xt_sb")
    for kd in range(KD):
        nc.sync.dma_start(out=xt_sb[:, kd, :], in_=xt[:, kd, :])
    w3_sb = consts.tile([128, KH, CP], dt_in, name="w3_sb")
    nc.sync.dma_start(out=w3_sb, in_=w3)
    oh_sb = consts.tile([C, NL], f32, name="oh_sb")
    nc.sync.dma_start(out=oh_sb, in_=oh)
    b1_sb = consts.tile([128, KH], f32, name="b1_sb")
    nc.sync.dma_start(out=b1_sb, in_=b1)
    b2_sb = consts.tile([128, KH], f32, name="b2_sb")
    nc.sync.dma_start(out=b2_sb, in_=b2)
    b3_sb = consts.tile([C, 1], f32, name="b3_sb")
    nc.sync.dma_start(out=b3_sb, in_=b3)
    ones_sb = consts.tile([C, 1], f32, name="ones_sb")
    nc.vector.memset(ones_sb, 1.0)

    h1_sb = acts.tile([128, KH, NL], dt_in, name="h1_sb")
    h2_sb = acts.tile([128, KH, NL], dt_in, name="h2_sb")

    def mm_layer(ps, w_t, rhs_sb, kt):
        """Accumulate ps[:, ns] += w_t[:, k].T @ rhs_sb[:, k, ns] over k."""
        for k in range(0, kt, kstep):
            for ns in range(NS):
                if fp8:
                    nc.tensor.matmul(
                        ps[:, ns * 512:(ns + 1) * 512],
                        lhsT=w_t[:, k:k + 2, :],
                        rhs=rhs_sb[:, k:k + 2, ns * 512:(ns + 1) * 512],
                        start=(k == 0),
                        stop=(k + 2 >= kt),
                        perf_mode=perf_mode,
                    )
                else:
                    nc.tensor.matmul(
                        ps[:, ns * 512:(ns + 1) * 512],
                        lhsT=w_t[:, k, :],
                        rhs=rhs_sb[:, k, ns * 512:(ns + 1) * 512],
                        start=(k == 0),
                        stop=(k + 1 >= kt),
                    )

    def finish_early():
        res = epil.tile([1, 1], f32, name="res", tag="res")
        nc.vector.reduce_sum(out=res, in_=h1_sb[0:1, 0, 0:128],
                             axis=mybir.AxisListType.X)
        nc.sync.dma_start(out=out, in_=res)

    if hw_loop and repeat > 1:
        reps = [0]
        loop_cm = tc.For_i(0, repeat, 1,
                           hint_engines=(mybir.EngineType.PE,))
    else:
        reps = range(repeat)
        loop_cm = contextlib.nullcontext()

    with loop_cm:
     for _rep in reps:
        # ---- Layer 1: h1 = relu(X @ W1.T + b1), stored as [j1, i] tiles
        for m in range(KH):
            w1_t = w1p.tile([128, KD, 128], dt_in, name="w1_t", tag="w1t")
            nc.sync.dma_start(out=w1_t, in_=w1[m])
            ps = psum.tile([128, NL], f32, name="ps1", tag="mm")
            mm_layer(ps, w1_t, xt_sb, KD)
            nc.scalar.activation(
                out=h1_sb[:, m, :], in_=ps,
                func=AF.Relu, bias=b1_sb[:, m:m + 1], scale=s12,
            )
        if stage == "l1":
            finish_early()
            continue

        # ---- Layer 2: h2 = relu(h1 @ W2.T + b2)
        for m in range(KH):
            w2_t = w2p.tile([128, KH, 128], dt_in, name="w2_t", tag="w2t")
            nc.sync.dma_start(out=w2_t, in_=w2[m])
            ps = psum.tile([128, NL], f32, name="ps2", tag="mm")
            mm_layer(ps, w2_t, h1_sb, KH)
            nc.scalar.activation(
                out=h2_sb[:, m, :], in_=ps,
                func=AF.Relu, bias=b2_sb[:, m:m + 1], scale=s12,
            )
        if stage == "l2":
            finish_early()
            continue

        # ---- Layer 3: logitsT[c, i] (pre-bias, scaled) in PSUM rows 0..15.
        # Rows 10..15 are zero-weight pad (classes padded to 16 so the
        # dual-fp8 DoubleRow pair-dim step is 16). In the "full2" variant
        # ps3 is split into two half-batch PSUM tensors so the first half's
        # evacuation/exp overlaps the second half's matmuls (separate
        # tensors avoid same-tensor PE-W/ACT-R serialization).
        lg = epil.tile([C, NL], f32, name="lg", tag="expT")
        expT = epil.tile([C, NL], f32, name="expT", tag="expT")
        halves = {"full1": 1, "full4": 4}.get(stage, 2)
        hw_cols = NL // halves
        for h in range(halves):
            ps3 = psum.tile([128, hw_cols], f32, name="ps3", tag="mm")
            hsl = slice(h * hw_cols, (h + 1) * hw_cols)
            for k in range(0, KH, kstep):
                for ns in range(hw_cols // 512):
                    col = h * hw_cols + ns * 512
                    if fp8:
                        nc.tensor.matmul(
                            ps3[0:CP, ns * 512:(ns + 1) * 512],
                            lhsT=w3_sb[:, k:k + 2, :],
                            rhs=h2_sb[:, k:k + 2, col:col + 512],
                            start=(k == 0),
                            stop=(k + 2 >= KH),
                            perf_mode=perf_mode,
                        )
                    else:
                        nc.tensor.matmul(
                            ps3[0:CP, ns * 512:(ns + 1) * 512],
                            lhsT=w3_sb[:, k, :],
                            rhs=h2_sb[:, k, col:col + 512],
                            start=(k == 0),
                            stop=(k + 1 >= KH),
                        )
            # lg = logitsT + b3 (scalar engine evacuates + rescales PSUM)
            nc.scalar.activation(out=lg[:, hsl], in_=ps3[0:C, :],
                                 func=AF.Identity, bias=b3_sb, scale=s3)
            # expT = exp(lg)
            nc.scalar.activation(out=expT[:, hsl], in_=lg[:, hsl],
                                 func=AF.Exp)

        # pick_b[c] = sum_i lg[c, i] * onehot[c, i] (in-place on lg; lg is
        # not needed afterwards)
        pick_b = epil.tile([C, 1], f32, name="pick_b", tag="pick")
        nc.vector.tensor_tensor(out=lg, in0=lg, in1=oh_sb,
                                op=mybir.AluOpType.mult)
        nc.vector.reduce_sum(out=pick_b, in_=lg, axis=mybir.AxisListType.X)

        # sumexp[1, i] via ones-matmuls over the class partitions
        pse = psum.tile([128, NL], f32, name="pse", tag="mm")
        for ns in range(NS):
            nc.tensor.matmul(
                pse[0:1, ns * 512:(ns + 1) * 512],
                lhsT=ones_sb,
                rhs=expT[:, ns * 512:(ns + 1) * 512],
                start=True, stop=True,
            )
        # lse_tot = sum_i log(sumexp_i), via the Ln activation's accumulator
        lse_tot = epil.tile([1, 1], f32, name="lse_tot", tag="lt")
        nc.scalar.activation(out=lg[0:1, :], in_=pse[0:1, :], func=AF.Ln,
                             accum_out=lse_tot)

        # totals: result = sum_c pick_b[c] - lse_tot
        pt_ps = psum.tile([128, 8], f32, name="pt_ps", tag="mm")
        nc.tensor.matmul(pt_ps[0:1, 0:1], lhsT=ones_sb, rhs=pick_b,
                         start=True, stop=True)
        res = epil.tile([1, 1], f32, name="res", tag="res")
        nc.vector.tensor_tensor(out=res, in0=pt_ps[0:1, 0:1], in1=lse_tot,
                                op=mybir.AluOpType.subtract)
        nc.sync.dma_start(out=out, in_=res)


def _build(repeat=1, stage="full", hw_loop=False, prec="fp8", wbufs=3):
    from contextlib import ExitStack

    import concourse.bacc as bacc
    import concourse.tile as tile
    from concourse import mybir

    f32 = mybir.dt.float32
    dt_in = mybir.dt.float8e4 if prec == "fp8" else mybir.dt.bfloat16

    nc = bacc.Bacc(
        "TRN2",
        target_bir_lowering=False,
        debug=False,
        enable_asserts=False,
        num_devices=N_CORES,
    )
    KD = D // 128
    KH = H // 128
    aps = {
        "xt": nc.dram_tensor("xt", [128, KD, NL], dt_in, kind="ExternalInput").ap(),
        "w1": nc.dram_tensor("w1", [KH, 128, KD, 128], dt_in, kind="ExternalInput").ap(),
        "w2": nc.dram_tensor("w2", [KH, 128, KH, 128], dt_in, kind="ExternalInput").ap(),
        "w3": nc.dram_tensor("w3", [128, KH, CP], dt_in, kind="ExternalInput").ap(),
        "b1": nc.dram_tensor("b1", [128, KH], f32, kind="ExternalInput").ap(),
        "b2": nc.dram_tensor("b2", [128, KH], f32, kind="ExternalInput").ap(),
        "b3": nc.dram_tensor("b3", [C, 1], f32, kind="ExternalInput").ap(),
        "oh": nc.dram_tensor("oh", [C, NL], f32, kind="ExternalInput").ap(),
        "out": nc.dram_tensor("out", [1, 1], f32, kind="ExternalOutput").ap(),
    }
    with tile.TileContext(nc) as tc:
        with ExitStack() as ctx:
            _emit(tc, ctx, aps, repeat, stage, hw_loop, prec, wbufs)
    nc.compile()
    return nc


def _q8(x, s):
    return np.clip(x.astype(np.float32) * s, -240.0, 240.0).astype(E4M3)


def prep_inputs(X, Y, W1, b1, W2, b2, W3, b3, prec="fp8"):
    """Shard + retile (+ scale/quantize) the full inputs into per-core maps."""
    KD = D // 128
    KH = H // 128
    fp8 = prec == "fp8"

    if fp8:
        W1c = _q8(W1, SW)
        W2c = _q8(W2, SW)
        W3c = _q8(W3, SW)
        b1c = (b1.astype(np.float32) * SH)
        b2c = (b2.astype(np.float32) * SH)
    else:
        W1c, W2c, W3c = W1.astype(BF16), W2.astype(BF16), W3.astype(BF16)
        b1c, b2c = b1.astype(np.float32), b2.astype(np.float32)

    w1p = np.ascontiguousarray(W1c.reshape(KH, 128, KD, 128).transpose(0, 3, 2, 1))
    w2p = np.ascontiguousarray(W2c.reshape(KH, 128, KH, 128).transpose(0, 3, 2, 1))
    W3pad = np.zeros((CP, H), dtype=W3c.dtype)
    W3pad[:C] = W3c
    w3p = np.ascontiguousarray(W3pad.reshape(CP, KH, 128).transpose(2, 1, 0))
    b1p = np.ascontiguousarray(b1c.reshape(KH, 128).T)
    b2p = np.ascontiguousarray(b2c.reshape(KH, 128).T)
    b3p = np.ascontiguousarray(b3.astype(np.float32).reshape(C, 1))

    Xb = _q8(X, SX) if fp8 else X.astype(BF16)
    in_maps = []
    for c in range(N_CORES):
        Xc = Xb[c * NL:(c + 1) * NL]
        xtp = np.ascontiguousarray(Xc.reshape(NL, KD, 128).transpose(2, 1, 0))
        Yc = Y[c * NL:(c + 1) * NL]
        ohp = (np.arange(C, dtype=np.int64)[:, None] == Yc[None, :].astype(np.int64))
        ohp = np.ascontiguousarray(ohp.astype(np.float32))
        in_maps.append({
            "xt": xtp, "w1": w1p, "w2": w2p, "w3": w3p,
            "b1": b1p, "b2": b2p, "b3": b3p, "oh": ohp,
        })
    return in_maps


def log_prior(W1, b1, W2, b2, W3, b3):
    params = (W1, b1, W2, b2, W3, b3)
    d = sum(p.size for p in params)
    sq = sum(float(np.sum(p.astype(np.float64) ** 2)) for p in params)
    return -0.5 * (sq / PRIOR_VAR + d * math.log(2.0 * math.pi * PRIOR_VAR))


def _get_nc(repeat=1, hw_loop=False, prec="fp8"):
    key = (repeat, hw_loop, prec)
    if key not in _compiled:
        _compiled[key] = _build(repeat, hw_loop=hw_loop, prec=prec)
    return _compiled[key]


def run_device(in_maps, repeat=1, prec="fp8"):
    from concourse.bass_utils import run_bass_kernel_spmd

    nc = _get_nc(repeat, prec=prec)
    res = run_bass_kernel_spmd(nc, in_maps, list(range(N_CORES)))
    return [r["out"][0, 0] for r in res.results]


def kernel(X, Y, W1, b1, W2, b2, W3, b3):
    X = np.asarray(X)
    Y = np.asarray(Y)
    W1 = np.asarray(W1)
    b1 = np.asarray(b1)
    W2 = np.asarray(W2)
    b2 = np.asarray(b2)
    W3 = np.asarray(W3)
    b3 = np.asarray(b3)

    try:
        in_maps = prep_inputs(X, Y, W1, b1, W2, b2, W3, b3, prec="fp8")
        partials = run_device(in_maps, prec="fp8")
    except Exception:
        # Safety net: fp8 DoubleRow leans on newer walrus/ISA behavior; the
        # bf16 path is plain matmuls.
        in_maps = prep_inputs(X, Y, W1, b1, W2, b2, W3, b3, prec="bf16")
        partials = run_device(in_maps, prec="bf16")
    total = float(np.sum(np.asarray(partials, dtype=np.float64)))
    total += log_prior(W1, b1, W2, b2, W3, b3)
    return np.array(total, dtype=np.float32)



# revision 9
# speedup vs baseline: 1.0730x; 1.0730x over previous
"""Trainium2 Bass kernel for the dense-MLP Bayesian log-joint problem.

Computes, for fixed MLP weights:
    h1 = relu(X @ W1.T + b1); h2 = relu(h1 @ W2.T + b2)
    logits = h2 @ W3.T + b3
    out = sum_i log_softmax(logits)[i, Y[i]] + log MVN(0, 100 I)(params)

Strategy: data-parallel over 8 NeuronCores. Each core gets 2048 rows of
X/Y plus a replicated copy of the (small) weights, computes its partial
log-likelihood sum on-device, and the host adds the partials plus the
closed-form Gaussian prior term.

On-device layout is "transposed activations": every matmul keeps the
contraction dim on SBUF partitions. The host pre-transposes X and the
weight matrices into PE-friendly tiles so no on-device transposes are
needed.

Matmuls run in fp8 (e4m3) with DoubleRow perf mode: inputs are scaled by
powers of two into fp8 range on the host, and the PSUM results are
rescaled exactly inside the (fp32) activation that applies bias+relu.
fp32 PSUM accumulation throughout; the log-softmax epilogue is fp32.

Perf structure (v2): all weights are DMA'd into SBUF once, outside the
steady-state loop. Each DoubleRow weight tile is loaded into the PE
array ONCE (a single InstLdweights) and then streamed against all four
512-column batch slices — the redundant per-matmul InstLdweights that
nc.tensor.matmul auto-emits are stripped post-emission. On HW this takes
the sustained matmul cadence from ~321 ns/MM (LDWEIGHTS-serialized) to
~101 ns/MM (microbenchmarked), i.e. the PE streams 2 output columns per
cycle in DoubleRow and next-tile weight loads overlap in the background
weight buffer.
"""

import math

import numpy as np
import ml_dtypes

N = 16384
D = 1024
H = 2048
C = 10
CP = 16  # classes padded to 16 so layer-3 DoubleRow satisfies step%16==0
N_CORES = 8
NL = N // N_CORES  # 2048 rows per core
PRIOR_VAR = 100.0

BF16 = ml_dtypes.bfloat16
E4M3 = ml_dtypes.float8_e4m3  # TRN fp8e4: max normal +-240

# Power-of-two scales that place X / weights / hidden activations into
# fp8e4m3's sweet spot. All rescales are exact in fp32.
SX = 16.0
SW = 128.0
SH = 16.0

_compiled = {}


def _strip_redundant_ldweights(nc):
    """Remove InstLdweights whose weights AP matches the most recent kept
    InstLdweights in the same block: the PE array already holds those
    weights, so the paired (non-self-loading) matmuls can reuse them.
    Waits on a stripped LDW migrate to the next kept instruction."""
    from concourse import mybir

    n_stripped = 0
    for b in nc.m.functions[0].blocks:
        insts = list(b.instructions)
        last_sig = None
        pending_waits = []
        for inst in insts:
            tn = type(inst).__name__
            if tn == "InstLdweights":
                ap = inst.ins[0]
                sig = (
                    ap.memref, ap.offset, str(ap.ap), str(ap.dtype),
                    str(getattr(inst, "perf_mode", None)),
                    str(getattr(inst, "is_transpose", None)),
                )
                if sig == last_sig:
                    si = inst.sync_info
                    if si is not None and si.on_wait:
                        pending_waits.extend(si.on_wait)
                    b.instructions.remove(inst)
                    n_stripped += 1
                    continue
                last_sig = sig
            elif tn in ("InstMatmult", "InstMatmultMx"):
                if getattr(inst, "is_transpose", False):
                    last_sig = None
            if pending_waits:
                si = inst.sync_info
                if si is None:
                    inst.sync_info = mybir.SyncInfo(
                        on_wait=list(pending_waits), on_update=[])
                else:
                    si.on_wait = list(si.on_wait) + pending_waits
                pending_waits = []
        assert not pending_waits
    return n_stripped


def _emit(tc, ctx, aps, repeat, stage="full", hw_loop=False, prec="fp8"):
    import contextlib

    from concourse import mybir

    nc = tc.nc
    f32 = mybir.dt.float32
    AF = mybir.ActivationFunctionType
    fp8 = prec == "fp8"
    dt_in = mybir.dt.float8e4 if fp8 else mybir.dt.bfloat16
    perf_mode = mybir.MatmulPerfMode.DoubleRow if fp8 else None
    kstep = 2 if fp8 else 1
    # PSUM -> activation rescales (exact powers of two)
    s12 = SH / (SX * SW) if fp8 else 1.0   # layer1 out scale; layer2 identical
    s3 = 1.0 / (SH * SW) if fp8 else 1.0

    xt, w1, w2, w3, b1, b2, b3, oh, out = (
        aps["xt"], aps["w1"], aps["w2"], aps["w3"],
        aps["b1"], aps["b2"], aps["b3"], aps["oh"], aps["out"],
    )

    KD = D // 128   # 8  k-tiles for layer 1
    KH = H // 128   # 16 k-tiles for layers 2/3, and m-tiles for layers 1/2
    NS = NL // 512  # 4  n-slices of the batch free dim

    consts = ctx.enter_context(tc.tile_pool(name="consts", bufs=1))
    acts = ctx.enter_context(tc.tile_pool(name="acts", bufs=1))
    psum = ctx.enter_context(tc.tile_pool(name="psum", bufs=2, space="PSUM"))
    epil = ctx.enter_context(tc.tile_pool(name="epil", bufs=2))

    # Constants / resident tensors (loaded once, outside the repeat loop)
    xt_sb = consts.tile([128, KD, NL], dt_in, name="xt_sb")
    for kd in range(KD):
        nc.sync.dma_start(out=xt_sb[:, kd, :], in_=xt[:, kd, :])
    if fp8:
        w1_sb = consts.tile([128, KH, KD, 128], dt_in, name="w1_sb")
        for m in range(KH):
            nc.sync.dma_start(out=w1_sb[:, m], in_=w1[m])
        w2_sb = consts.tile([128, KH, KH, 128], dt_in, name="w2_sb")
        for m in range(KH):
            nc.sync.dma_start(out=w2_sb[:, m], in_=w2[m])

        def w1_of(m0):
            return w1_sb[:, m0], w1_sb[:, m0 + 1]

        def w2_of(m0):
            return w2_sb[:, m0], w2_sb[:, m0 + 1]
    else:
        # bf16 safety-net: tensors are 2x the fp8 size, so weights stream
        # from DRAM per m-tile (v1 structure; slower but correct).
        w1p = ctx.enter_context(tc.tile_pool(name="w1p", bufs=2))
        w2p = ctx.enter_context(tc.tile_pool(name="w2p", bufs=1))

        def w1_of(m0):
            wt = w1p.tile([128, 2, KD, 128], dt_in, name="w1_t", tag="w1t")
            nc.sync.dma_start(out=wt[:, 0], in_=w1[m0])
            nc.sync.dma_start(out=wt[:, 1], in_=w1[m0 + 1])
            return wt[:, 0], wt[:, 1]

        def w2_of(m0):
            wt = w2p.tile([128, 2, KH, 128], dt_in, name="w2_t", tag="w2t")
            nc.sync.dma_start(out=wt[:, 0], in_=w2[m0])
            nc.sync.dma_start(out=wt[:, 1], in_=w2[m0 + 1])
            return wt[:, 0], wt[:, 1]
    w3_sb = consts.tile([128, KH, CP], dt_in, name="w3_sb")
    nc.sync.dma_start(out=w3_sb, in_=w3)
    oh_sb = consts.tile([C, NL], f32, name="oh_sb")
    nc.sync.dma_start(out=oh_sb, in_=oh)
    b1_sb = consts.tile([128, KH], f32, name="b1_sb")
    nc.sync.dma_start(out=b1_sb, in_=b1)
    b2_sb = consts.tile([128, KH], f32, name="b2_sb")
    nc.sync.dma_start(out=b2_sb, in_=b2)
    b3_sb = consts.tile([C, 1], f32, name="b3_sb")
    nc.sync.dma_start(out=b3_sb, in_=b3)
    ones_sb = consts.tile([C, 1], f32, name="ones_sb")
    nc.vector.memset(ones_sb, 1.0)

    h1_sb = acts.tile([128, KH, NL], dt_in, name="h1_sb")
    h2_sb = acts.tile([128, KH, NL], dt_in, name="h2_sb")

    def mm_block(ps, w_t, rhs_sb, k, kt, cols=NL, col0=0):
        """One k-group: 4 matmuls over the 512-wide batch slices, sharing
        one stationary weight load (redundant LDWs stripped)."""
        for ns in range(cols // 512):
            c = col0 + ns * 512
            if fp8:
                nc.tensor.matmul(
                    ps[:, c - col0:c - col0 + 512] if col0 else ps[:, c:c + 512],
                    lhsT=w_t[:, k:k + 2, :],
                    rhs=rhs_sb[:, k:k + 2, c:c + 512],
                    start=(k == 0),
                    stop=(k + 2 >= kt),
                    perf_mode=perf_mode,
                )
            else:
                nc.tensor.matmul(
                    ps[:, c - col0:c - col0 + 512] if col0 else ps[:, c:c + 512],
                    lhsT=w_t[:, k, :],
                    rhs=rhs_sb[:, k, c:c + 512],
                    start=(k == 0),
                    stop=(k + 1 >= kt),
                )

    def mm_pair(w_of, m0, rhs_sb, kt, out_sb, b_sb):
        """Two m-tiles interleaved kgroup-wise over two PSUM buffers so
        consecutive 4-MM blocks alternate bank sets (PSUM bank-reuse gap 8:
        revisiting a bank within ~4 MMs of its last write stalls the PE)."""
        w0, w1t = w_of(m0)
        ps_a = psum.tile([128, NL], f32, name="ps_a", tag="mm")
        ps_b = psum.tile([128, NL], f32, name="ps_b", tag="mm")
        for k in range(0, kt, kstep):
            mm_block(ps_a, w0, rhs_sb, k, kt)
            mm_block(ps_b, w1t, rhs_sb, k, kt)
        for mi, ps in ((0, ps_a), (1, ps_b)):
            nc.scalar.activation(
                out=out_sb[:, m0 + mi, :], in_=ps,
                func=AF.Relu, bias=b_sb[:, m0 + mi:m0 + mi + 1], scale=s12,
            )

    def finish_early():
        res = epil.tile([1, 1], f32, name="res", tag="res")
        nc.vector.reduce_sum(out=res, in_=h1_sb[0:1, 0, 0:128],
                             axis=mybir.AxisListType.X)
        nc.sync.dma_start(out=out, in_=res)

    if hw_loop and repeat > 1:
        reps = [0]
        loop_cm = tc.For_i(0, repeat, 1,
                           hint_engines=(mybir.EngineType.PE,))
    else:
        reps = range(repeat)
        loop_cm = contextlib.nullcontext()

    with loop_cm:
     for _rep in reps:
        # ---- Layer 1: h1 = relu(X @ W1.T + b1), stored as [j1, i] tiles
        for m in range(0, KH, 2):
            mm_pair(w1_of, m, xt_sb, KD, h1_sb, b1_sb)
        if stage == "l1":
            finish_early()
            continue

        # ---- Layer 2: h2 = relu(h1 @ W2.T + b2)
        for m in range(0, KH, 2):
            mm_pair(w2_of, m, h1_sb, KH, h2_sb, b2_sb)
        if stage == "l2":
            finish_early()
            continue

        # ---- Layer 3: logitsT[c, i] (pre-bias, scaled) in PSUM rows 0..15.
        # Rows 10..15 are zero-weight pad (classes padded to 16 so the
        # dual-fp8 DoubleRow pair-dim step is 16). The k-contraction is
        # split into two chains (even kgroups -> ps3a, odd -> ps3b) so
        # consecutive 4-MM blocks alternate PSUM bank sets; the vector
        # engine sums the two partials in PSUM before the bias/exp epilogue.
        lg = epil.tile([C, NL], f32, name="lg", tag="expT")
        expT = epil.tile([C, NL], f32, name="expT", tag="expT")
        ps3a = psum.tile([128, NL], f32, name="ps3a", tag="mm")
        nkg = KH // kstep
        if fp8:
            ps3b = psum.tile([128, NL], f32, name="ps3b", tag="mm")
            for kg in range(nkg):
                k = kg * kstep
                ps3 = ps3a if kg % 2 == 0 else ps3b
                for ns in range(NS):
                    col = ns * 512
                    nc.tensor.matmul(
                        ps3[0:CP, col:col + 512],
                        lhsT=w3_sb[:, k:k + 2, :],
                        rhs=h2_sb[:, k:k + 2, col:col + 512],
                        start=kg < 2,
                        stop=kg >= nkg - 2,
                        perf_mode=perf_mode,
                    )
            # Fold chain B into chain A: ACT evacuates ps3b to SBUF (raw),
            # then DVE adds it into ps3a (only one PSUM input per DVE op).
            lgb = epil.tile([C, NL], f32, name="lgb", tag="lgb")
            nc.scalar.activation(out=lgb, in_=ps3b[0:C, :], func=AF.Identity)
            nc.vector.tensor_tensor(out=ps3a[0:C, :], in0=ps3a[0:C, :],
                                    in1=lgb, op=mybir.AluOpType.add)
        else:
            for kg in range(nkg):
                k = kg * kstep
                for ns in range(NS):
                    col = ns * 512
                    nc.tensor.matmul(
                        ps3a[0:CP, col:col + 512],
                        lhsT=w3_sb[:, k, :],
                        rhs=h2_sb[:, k, col:col + 512],
                        start=kg == 0,
                        stop=kg >= nkg - 1,
                    )
        # lg = logitsT + b3 (scalar engine evacuates + rescales PSUM)
        nc.scalar.activation(out=lg, in_=ps3a[0:C, :],
                             func=AF.Identity, bias=b3_sb, scale=s3)
        # expT = exp(lg)
        nc.scalar.activation(out=expT, in_=lg, func=AF.Exp)

        # pick_b[c] = sum_i lg[c, i] * onehot[c, i] (in-place on lg; lg is
        # not needed afterwards)
        pick_b = epil.tile([C, 1], f32, name="pick_b", tag="pick")
        nc.vector.tensor_tensor(out=lg, in0=lg, in1=oh_sb,
                                op=mybir.AluOpType.mult)
        nc.vector.reduce_sum(out=pick_b, in_=lg, axis=mybir.AxisListType.X)

        # sumexp[1, i] via ones-matmuls over the class partitions
        pse = psum.tile([128, NL], f32, name="pse", tag="mm")
        for ns in range(NS):
            nc.tensor.matmul(
                pse[0:1, ns * 512:(ns + 1) * 512],
                lhsT=ones_sb,
                rhs=expT[:, ns * 512:(ns + 1) * 512],
                start=True, stop=True,
            )
        # lse_tot = sum_i log(sumexp_i), via the Ln activation's accumulator
        lse_tot = epil.tile([1, 1], f32, name="lse_tot", tag="lt")
        nc.scalar.activation(out=lg[0:1, :], in_=pse[0:1, :], func=AF.Ln,
                             accum_out=lse_tot)

        # totals: result = sum_c pick_b[c] - lse_tot
        pt_ps = psum.tile([128, 8], f32, name="pt_ps", tag="mm")
        nc.tensor.matmul(pt_ps[0:1, 0:1], lhsT=ones_sb, rhs=pick_b,
                         start=True, stop=True)
        res = epil.tile([1, 1], f32, name="res", tag="res")
        nc.vector.tensor_tensor(out=res, in0=pt_ps[0:1, 0:1], in1=lse_tot,
                                op=mybir.AluOpType.subtract)
        nc.sync.dma_start(out=out, in_=res)


def _build(repeat=1, stage="full", hw_loop=False, prec="fp8"):
    from contextlib import ExitStack

    import concourse.bacc as bacc
    import concourse.tile as tile
    from concourse import mybir

    f32 = mybir.dt.float32
    dt_in = mybir.dt.float8e4 if prec == "fp8" else mybir.dt.bfloat16

    nc = bacc.Bacc(
        "TRN2",
        target_bir_lowering=False,
        debug=False,
        enable_asserts=False,
        num_devices=N_CORES,
    )
    KD = D // 128
    KH = H // 128
    aps = {
        "xt": nc.dram_tensor("xt", [128, KD, NL], dt_in, kind="ExternalInput").ap(),
        "w1": nc.dram_tensor("w1", [KH, 128, KD, 128], dt_in, kind="ExternalInput").ap(),
        "w2": nc.dram_tensor("w2", [KH, 128, KH, 128], dt_in, kind="ExternalInput").ap(),
        "w3": nc.dram_tensor("w3", [128, KH, CP], dt_in, kind="ExternalInput").ap(),
        "b1": nc.dram_tensor("b1", [128, KH], f32, kind="ExternalInput").ap(),
        "b2": nc.dram_tensor("b2", [128, KH], f32, kind="ExternalInput").ap(),
        "b3": nc.dram_tensor("b3", [C, 1], f32, kind="ExternalInput").ap(),
        "oh": nc.dram_tensor("oh", [C, NL], f32, kind="ExternalInput").ap(),
        "out": nc.dram_tensor("out", [1, 1], f32, kind="ExternalOutput").ap(),
    }
    with tile.TileContext(nc) as tc:
        with ExitStack() as ctx:
            _emit(tc, ctx, aps, repeat, stage, hw_loop, prec)
    _strip_redundant_ldweights(nc)
    nc.compile()
    return nc


def _q8(x, s):
    return np.clip(x.astype(np.float32) * s, -240.0, 240.0).astype(E4M3)


def prep_inputs(X, Y, W1, b1, W2, b2, W3, b3, prec="fp8"):
    """Shard + retile (+ scale/quantize) the full inputs into per-core maps."""
    KD = D // 128
    KH = H // 128
    fp8 = prec == "fp8"

    if fp8:
        W1c = _q8(W1, SW)
        W2c = _q8(W2, SW)
        W3c = _q8(W3, SW)
        b1c = (b1.astype(np.float32) * SH)
        b2c = (b2.astype(np.float32) * SH)
    else:
        W1c, W2c, W3c = W1.astype(BF16), W2.astype(BF16), W3.astype(BF16)
        b1c, b2c = b1.astype(np.float32), b2.astype(np.float32)

    w1p = np.ascontiguousarray(W1c.reshape(KH, 128, KD, 128).transpose(0, 3, 2, 1))
    w2p = np.ascontiguousarray(W2c.reshape(KH, 128, KH, 128).transpose(0, 3, 2, 1))
    W3pad = np.zeros((CP, H), dtype=W3c.dtype)
    W3pad[:C] = W3c
    w3p = np.ascontiguousarray(W3pad.reshape(CP, KH, 128).transpose(2, 1, 0))
    b1p = np.ascontiguousarray(b1c.reshape(KH, 128).T)
    b2p = np.ascontiguousarray(b2c.reshape(KH, 128).T)
    b3p = np.ascontiguousarray(b3.astype(np.float32).reshape(C, 1))

    Xb = _q8(X, SX) if fp8 else X.astype(BF16)
    in_maps = []
    for c in range(N_CORES):
        Xc = Xb[c * NL:(c + 1) * NL]
        xtp = np.ascontiguousarray(Xc.reshape(NL, KD, 128).transpose(2, 1, 0))
        Yc = Y[c * NL:(c + 1) * NL]
        ohp = (np.arange(C, dtype=np.int64)[:, None] == Yc[None, :].astype(np.int64))
        ohp = np.ascontiguousarray(ohp.astype(np.float32))
        in_maps.append({
            "xt": xtp, "w1": w1p, "w2": w2p, "w3": w3p,
            "b1": b1p, "b2": b2p, "b3": b3p, "oh": ohp,
        })
    return in_maps


def log_prior(W1, b1, W2, b2, W3, b3):
    params = (W1, b1, W2, b2, W3, b3)
    d = sum(p.size for p in params)
    sq = sum(float(np.sum(p.astype(np.float64) ** 2)) for p in params)
    return -0.5 * (sq / PRIOR_VAR + d * math.log(2.0 * math.pi * PRIOR_VAR))


def _get_nc(repeat=1, hw_loop=False, prec="fp8"):
    key = (repeat, hw_loop, prec)
    if key not in _compiled:
        _compiled[key] = _build(repeat, hw_loop=hw_loop, prec=prec)
    return _compiled[key]


def run_device(in_maps, repeat=1, prec="fp8"):
    from concourse.bass_utils import run_bass_kernel_spmd

    nc = _get_nc(repeat, prec=prec)
    res = run_bass_kernel_spmd(nc, in_maps, list(range(N_CORES)))
    return [r["out"][0, 0] for r in res.results]


def kernel(X, Y, W1, b1, W2, b2, W3, b3):
    X = np.asarray(X)
    Y = np.asarray(Y)
    W1 = np.asarray(W1)
    b1 = np.asarray(b1)
    W2 = np.asarray(W2)
    b2 = np.asarray(b2)
    W3 = np.asarray(W3)
    b3 = np.asarray(b3)

    try:
        in_maps = prep_inputs(X, Y, W1, b1, W2, b2, W3, b3, prec="fp8")
        partials = run_device(in_maps, prec="fp8")
    except Exception:
        # Safety net: fp8 DoubleRow leans on newer walrus/ISA behavior; the
        # bf16 path is plain matmuls.
        in_maps = prep_inputs(X, Y, W1, b1, W2, b2, W3, b3, prec="bf16")
        partials = run_device(in_maps, prec="bf16")
    total = float(np.sum(np.asarray(partials, dtype=np.float64)))
    total += log_prior(W1, b1, W2, b2, W3, b3)
    return np.array(total, dtype=np.float32)


# revision 12
# speedup vs baseline: 1.2695x; 1.1832x over previous
"""Trainium2 Bass kernel for the dense-MLP Bayesian log-joint problem.

Computes, for fixed MLP weights:
    h1 = relu(X @ W1.T + b1); h2 = relu(h1 @ W2.T + b2)
    logits = h2 @ W3.T + b3
    out = sum_i log_softmax(logits)[i, Y[i]] + log MVN(0, 100 I)(params)

Strategy: data-parallel over 8 NeuronCores. Each core gets 2048 rows of
X/Y plus a replicated copy of the (small) weights, computes its partial
log-likelihood sum on-device, and the host adds the partials plus the
closed-form Gaussian prior term.

On-device layout is "transposed activations": every matmul keeps the
contraction dim on SBUF partitions. The host pre-transposes X and the
weight matrices into PE-friendly tiles so no on-device transposes are
needed.

Matmuls run in fp8 (e4m3) with DoubleRow perf mode: inputs are scaled by
powers of two into fp8 range on the host, and the PSUM results are
rescaled exactly inside the (fp32) activation that applies bias+relu.
fp32 PSUM accumulation throughout; the log-softmax epilogue is fp32.

Perf structure (v2): all weights are DMA'd into SBUF once, outside the
steady-state loop. Each DoubleRow weight tile is loaded into the PE
array ONCE (a single InstLdweights) and then streamed against all four
512-column batch slices — the redundant per-matmul InstLdweights that
nc.tensor.matmul auto-emits are stripped post-emission. On HW this takes
the sustained matmul cadence from ~321 ns/MM (LDWEIGHTS-serialized) to
~101 ns/MM (microbenchmarked), i.e. the PE streams 2 output columns per
cycle in DoubleRow and next-tile weight loads overlap in the background
weight buffer.
"""

import math

import numpy as np
import ml_dtypes

N = 16384
D = 1024
H = 2048
C = 10
CP = 16  # classes padded to 16 so layer-3 DoubleRow satisfies step%16==0
N_CORES = 8
NL = N // N_CORES  # 2048 rows per core
PRIOR_VAR = 100.0

BF16 = ml_dtypes.bfloat16
E4M3 = ml_dtypes.float8_e4m3  # TRN fp8e4: max normal +-240

# Power-of-two scales that place X / weights / hidden activations into
# fp8e4m3's sweet spot. All rescales are exact in fp32.
SX = 16.0
SW = 128.0
SH = 16.0

_compiled = {}


def _strip_redundant_ldweights(nc):
    """Remove InstLdweights whose weights AP matches the most recent kept
    InstLdweights in the same block: the PE array already holds those
    weights, so the paired (non-self-loading) matmuls can reuse them.
    Waits on a stripped LDW migrate to the next kept instruction."""
    from concourse import mybir

    n_stripped = 0
    for b in nc.m.functions[0].blocks:
        insts = list(b.instructions)
        last_sig = None
        pending_waits = []
        for inst in insts:
            tn = type(inst).__name__
            if tn == "InstLdweights":
                ap = inst.ins[0]
                sig = (
                    ap.memref, ap.offset, str(ap.ap), str(ap.dtype),
                    str(getattr(inst, "perf_mode", None)),
                    str(getattr(inst, "is_transpose", None)),
                )
                if sig == last_sig:
                    si = inst.sync_info
                    if si is not None and si.on_wait:
                        pending_waits.extend(si.on_wait)
                    b.instructions.remove(inst)
                    n_stripped += 1
                    continue
                last_sig = sig
            elif tn in ("InstMatmult", "InstMatmultMx"):
                if getattr(inst, "is_transpose", False):
                    last_sig = None
            if pending_waits:
                si = inst.sync_info
                if si is None:
                    inst.sync_info = mybir.SyncInfo(
                        on_wait=list(pending_waits), on_update=[])
                else:
                    si.on_wait = list(si.on_wait) + pending_waits
                pending_waits = []
        assert not pending_waits
    return n_stripped


def _emit(tc, ctx, aps, repeat, stage="full", hw_loop=False, prec="fp8"):
    import contextlib

    from concourse import mybir

    nc = tc.nc
    f32 = mybir.dt.float32
    AF = mybir.ActivationFunctionType
    fp8 = prec == "fp8"
    dt_in = mybir.dt.float8e4 if fp8 else mybir.dt.bfloat16
    perf_mode = mybir.MatmulPerfMode.DoubleRow if fp8 else None
    kstep = 2 if fp8 else 1
    # PSUM -> activation rescales (exact powers of two)
    s12 = SH / (SX * SW) if fp8 else 1.0   # layer1 out scale; layer2 identical
    s3 = 1.0 / (SH * SW) if fp8 else 1.0

    xt, w1, w2, w3, b1, b2, b3, oh, out = (
        aps["xt"], aps["w1"], aps["w2"], aps["w3"],
        aps["b1"], aps["b2"], aps["b3"], aps["oh"], aps["out"],
    )

    KD = D // 128   # 8  k-tiles for layer 1
    KH = H // 128   # 16 k-tiles for layers 2/3, and m-tiles for layers 1/2
    NS = NL // 512  # 4  n-slices of the batch free dim

    consts = ctx.enter_context(tc.tile_pool(name="consts", bufs=1))
    psw = ctx.enter_context(tc.tile_pool(name="psw", bufs=4, space="PSUM"))
    acts = ctx.enter_context(tc.tile_pool(name="acts", bufs=1))
    psum = ctx.enter_context(tc.tile_pool(name="psum", bufs=2, space="PSUM"))
    epil = ctx.enter_context(tc.tile_pool(name="epil", bufs=2))

    # Constants / resident tensors (loaded once, outside the repeat loop)
    xt_sb = consts.tile([128, KD, NL], dt_in, name="xt_sb")
    for kd in range(KD):
        nc.sync.dma_start(out=xt_sb[:, kd, :], in_=xt[:, kd, :])
    if fp8:
        w1_sb = consts.tile([128, KH, KD, 128], dt_in, name="w1_sb")
        for m in range(KH):
            nc.sync.dma_start(out=w1_sb[:, m], in_=w1[m])
        w2_sb = consts.tile([128, KH, KH, 128], dt_in, name="w2_sb")
        for m in range(KH):
            nc.sync.dma_start(out=w2_sb[:, m], in_=w2[m])

        def w1_of(m0):
            return w1_sb[:, m0], w1_sb[:, m0 + 1]

        def w2_of(m0):
            return w2_sb[:, m0], w2_sb[:, m0 + 1]
    else:
        # bf16 safety-net: tensors are 2x the fp8 size, so weights stream
        # from DRAM per m-tile (v1 structure; slower but correct).
        w1p = ctx.enter_context(tc.tile_pool(name="w1p", bufs=2))
        w2p = ctx.enter_context(tc.tile_pool(name="w2p", bufs=1))

        def w1_of(m0):
            wt = w1p.tile([128, 2, KD, 128], dt_in, name="w1_t", tag="w1t")
            nc.sync.dma_start(out=wt[:, 0], in_=w1[m0])
            nc.sync.dma_start(out=wt[:, 1], in_=w1[m0 + 1])
            return wt[:, 0], wt[:, 1]

        def w2_of(m0):
            wt = w2p.tile([128, 2, KH, 128], dt_in, name="w2_t", tag="w2t")
            nc.sync.dma_start(out=wt[:, 0], in_=w2[m0])
            nc.sync.dma_start(out=wt[:, 1], in_=w2[m0 + 1])
            return wt[:, 0], wt[:, 1]
    w3_sb = consts.tile([128, KH, CP], dt_in, name="w3_sb")
    nc.sync.dma_start(out=w3_sb, in_=w3)
    oh_sb = consts.tile([C, NL], f32, name="oh_sb")
    nc.sync.dma_start(out=oh_sb, in_=oh)
    b1_sb = consts.tile([128, KH], f32, name="b1_sb")
    nc.sync.dma_start(out=b1_sb, in_=b1)
    b2_sb = consts.tile([128, KH], f32, name="b2_sb")
    nc.sync.dma_start(out=b2_sb, in_=b2)
    b3_sb = consts.tile([C, 1], f32, name="b3_sb")
    nc.sync.dma_start(out=b3_sb, in_=b3)
    ones_sb = consts.tile([C, 1], f32, name="ones_sb")
    nc.vector.memset(ones_sb, 1.0)

    h1_sb = acts.tile([128, KH, NL], dt_in, name="h1_sb")
    h2_sb = acts.tile([128, KH, NL], dt_in, name="h2_sb")
    h1c = None
    if stage == "l2x":
        # Diagnostic: static stand-in for h1 so L2's rhs is not ACT-written
        h1c = consts.tile([128, KH, NL], dt_in, name="h1c")
        nc.vector.memset(h1c, 0.03)

    def mm_block(ps, w_t, rhs_sb, k, kt, cols=NL, col0=0):
        """One k-group: 4 matmuls over the 512-wide batch slices, sharing
        one stationary weight load (redundant LDWs stripped)."""
        for ns in range(cols // 512):
            c = col0 + ns * 512
            if fp8:
                nc.tensor.matmul(
                    ps[:, c - col0:c - col0 + 512] if col0 else ps[:, c:c + 512],
                    lhsT=w_t[:, k:k + 2, :],
                    rhs=rhs_sb[:, k:k + 2, c:c + 512],
                    start=(k == 0),
                    stop=(k + 2 >= kt),
                    perf_mode=perf_mode,
                )
            else:
                nc.tensor.matmul(
                    ps[:, c - col0:c - col0 + 512] if col0 else ps[:, c:c + 512],
                    lhsT=w_t[:, k, :],
                    rhs=rhs_sb[:, k, c:c + 512],
                    start=(k == 0),
                    stop=(k + 1 >= kt),
                )

    def mm_pair(w_of, m0, rhs_sb, kt, out_sb, b_sb, act=True):
        """Two m-tiles interleaved kgroup-wise over two PSUM buffers so
        consecutive 4-MM blocks alternate bank sets (PSUM bank-reuse gap 8:
        revisiting a bank within ~4 MMs of its last write stalls the PE)."""
        w0, w1t = w_of(m0)
        ps_a = psum.tile([128, NL], f32, name="ps_a", tag="mm")
        ps_b = psum.tile([128, NL], f32, name="ps_b", tag="mm")
        for k in range(0, kt, kstep):
            mm_block(ps_a, w0, rhs_sb, k, kt)
            mm_block(ps_b, w1t, rhs_sb, k, kt)
        if not act:
            return
        for mi, ps in ((0, ps_a), (1, ps_b)):
            nc.scalar.activation(
                out=out_sb[:, m0 + mi, :], in_=ps,
                func=AF.Relu, bias=b_sb[:, m0 + mi:m0 + mi + 1], scale=s12,
            )

    def mm_window3(w_of, rhs_sb, kt, out_sb, b_sb):
        """Round-robin window of 3 live half-batch chains on [128, 1024]
        PSUM tiles (6 banks busy, 2 free for the evacuating tile), so ACT
        overlaps the PE stream and bank-reuse gap is 6 MMs."""
        nkg = kt // kstep
        halves = [(m, h) for m in range(KH) for h in range(2)]
        wcache = {}

        def w_for(m):
            if m not in wcache:
                p0 = m & ~1
                pair = w_of(p0)
                wcache[p0] = pair[0]
                wcache[p0 + 1] = pair[1]
            return wcache[m]

        W = 3
        active = []  # (m, h, tile, kg_done)
        nexti = 0
        while active or nexti < len(halves):
            while len(active) < W and nexti < len(halves):
                m, h = halves[nexti]
                nexti += 1
                t = psw.tile([128, 1024], f32, name="psw_t", tag="w3")
                active.append([m, h, t, 0])
                # stagger chain starts so completions spread out
                if len(active) < W and nexti <= W:
                    break
            retired = []
            for ent in list(active):
                m, h, t, kg = ent
                k = kg * kstep
                mm_block(t, w_for(m), rhs_sb, k, kt,
                         cols=1024, col0=h * 1024)
                ent[3] += 1
                if ent[3] * kstep >= kt:
                    nc.scalar.activation(
                        out=out_sb[:, m, h * 1024:(h + 1) * 1024],
                        in_=t, func=AF.Relu,
                        bias=b_sb[:, m:m + 1], scale=s12,
                    )
                    retired.append(ent)
            for ent in retired:
                active.remove(ent)

    def finish_early():
        res = epil.tile([1, 1], f32, name="res", tag="res")
        nc.vector.reduce_sum(out=res, in_=h1_sb[0:1, 0, 0:128],
                             axis=mybir.AxisListType.X)
        nc.sync.dma_start(out=out, in_=res)

    if hw_loop and repeat > 1:
        reps = [0]
        loop_cm = tc.For_i(0, repeat, 1,
                           hint_engines=(mybir.EngineType.PE,))
    else:
        reps = range(repeat)
        loop_cm = contextlib.nullcontext()

    with loop_cm:
     for _rep in reps:
        # ---- Layer 1: h1 = relu(X @ W1.T + b1), stored as [j1, i] tiles
        if stage == "l2w3":
            mm_window3(w1_of, xt_sb, KD, h1_sb, b1_sb)
        else:
            for m in range(0, KH, 2):
                mm_pair(w1_of, m, xt_sb, KD, h1_sb, b1_sb)
        if stage == "l1":
            finish_early()
            continue

        # ---- Layer 2: h2 = relu(h1 @ W2.T + b2)
        rhs2 = h1c if stage == "l2x" else h1_sb
        if stage == "l2w3":
            mm_window3(w2_of, rhs2, KH, h2_sb, b2_sb)
        else:
            for m in range(0, KH, 2):
                mm_pair(w2_of, m, rhs2, KH, h2_sb, b2_sb,
                        act=(stage != "l2noact"))
        if stage in ("l2", "l2x", "l2noact", "l2w3"):
            finish_early()
            continue

        # ---- Layer 3: logitsT[c, i] (pre-bias, scaled) in PSUM rows 0..15.
        # Rows 10..15 are zero-weight pad (classes padded to 16 so the
        # dual-fp8 DoubleRow pair-dim step is 16). The k-contraction is
        # split into two chains (even kgroups -> ps3a, odd -> ps3b) so
        # consecutive 4-MM blocks alternate PSUM bank sets; the vector
        # engine sums the two partials in PSUM before the bias/exp epilogue.
        lg = epil.tile([C, NL], f32, name="lg", tag="expT")
        expT = epil.tile([C, NL], f32, name="expT", tag="expT")
        ps3a = psum.tile([128, NL], f32, name="ps3a", tag="mm")
        nkg = KH // kstep
        if fp8:
            ps3b = psum.tile([128, NL], f32, name="ps3b", tag="mm")
            for kg in range(nkg):
                k = kg * kstep
                ps3 = ps3a if kg % 2 == 0 else ps3b
                for ns in range(NS):
                    col = ns * 512
                    nc.tensor.matmul(
                        ps3[0:CP, col:col + 512],
                        lhsT=w3_sb[:, k:k + 2, :],
                        rhs=h2_sb[:, k:k + 2, col:col + 512],
                        start=kg < 2,
                        stop=kg >= nkg - 2,
                        perf_mode=perf_mode,
                    )
            # Fold chain B into chain A: ACT evacuates ps3b to SBUF (raw),
            # then DVE adds it into ps3a (only one PSUM input per DVE op).
            lgb = epil.tile([C, NL], f32, name="lgb", tag="lgb")
            nc.scalar.activation(out=lgb, in_=ps3b[0:C, :], func=AF.Identity)
            nc.vector.tensor_tensor(out=ps3a[0:C, :], in0=ps3a[0:C, :],
                                    in1=lgb, op=mybir.AluOpType.add)
        else:
            for kg in range(nkg):
                k = kg * kstep
                for ns in range(NS):
                    col = ns * 512
                    nc.tensor.matmul(
                        ps3a[0:CP, col:col + 512],
                        lhsT=w3_sb[:, k, :],
                        rhs=h2_sb[:, k, col:col + 512],
                        start=kg == 0,
                        stop=kg >= nkg - 1,
                    )
        # lg = logitsT + b3 (scalar engine evacuates + rescales PSUM)
        nc.scalar.activation(out=lg, in_=ps3a[0:C, :],
                             func=AF.Identity, bias=b3_sb, scale=s3)
        # expT = exp(lg)
        nc.scalar.activation(out=expT, in_=lg, func=AF.Exp)

        # pick_b[c] = sum_i lg[c, i] * onehot[c, i] (in-place on lg; lg is
        # not needed afterwards)
        pick_b = epil.tile([C, 1], f32, name="pick_b", tag="pick")
        nc.vector.tensor_tensor(out=lg, in0=lg, in1=oh_sb,
                                op=mybir.AluOpType.mult)
        nc.vector.reduce_sum(out=pick_b, in_=lg, axis=mybir.AxisListType.X)

        # sumexp[1, i] via ones-matmuls over the class partitions
        pse = psum.tile([128, NL], f32, name="pse", tag="mm")
        for ns in range(NS):
            nc.tensor.matmul(
                pse[0:1, ns * 512:(ns + 1) * 512],
                lhsT=ones_sb,
                rhs=expT[:, ns * 512:(ns + 1) * 512],
                start=True, stop=True,
            )
        # lse_tot = sum_i log(sumexp_i), via the Ln activation's accumulator
        lse_tot = epil.tile([1, 1], f32, name="lse_tot", tag="lt")
        nc.scalar.activation(out=lg[0:1, :], in_=pse[0:1, :], func=AF.Ln,
                             accum_out=lse_tot)

        # totals: result = sum_c pick_b[c] - lse_tot
        pt_ps = psum.tile([128, 8], f32, name="pt_ps", tag="mm")
        nc.tensor.matmul(pt_ps[0:1, 0:1], lhsT=ones_sb, rhs=pick_b,
                         start=True, stop=True)
        res = epil.tile([1, 1], f32, name="res", tag="res")
        nc.vector.tensor_tensor(out=res, in0=pt_ps[0:1, 0:1], in1=lse_tot,
                                op=mybir.AluOpType.subtract)
        nc.sync.dma_start(out=out, in_=res)


def _build(repeat=1, stage="full", hw_loop=False, prec="fp8"):
    from contextlib import ExitStack

    import concourse.bacc as bacc
    import concourse.tile as tile
    from concourse import mybir

    f32 = mybir.dt.float32
    dt_in = mybir.dt.float8e4 if prec == "fp8" else mybir.dt.bfloat16

    nc = bacc.Bacc(
        "TRN2",
        target_bir_lowering=False,
        debug=False,
        enable_asserts=False,
        num_devices=N_CORES,
    )
    KD = D // 128
    KH = H // 128
    aps = {
        "xt": nc.dram_tensor("xt", [128, KD, NL], dt_in, kind="ExternalInput").ap(),
        "w1": nc.dram_tensor("w1", [KH, 128, KD, 128], dt_in, kind="ExternalInput").ap(),
        "w2": nc.dram_tensor("w2", [KH, 128, KH, 128], dt_in, kind="ExternalInput").ap(),
        "w3": nc.dram_tensor("w3", [128, KH, CP], dt_in, kind="ExternalInput").ap(),
        "b1": nc.dram_tensor("b1", [128, KH], f32, kind="ExternalInput").ap(),
        "b2": nc.dram_tensor("b2", [128, KH], f32, kind="ExternalInput").ap(),
        "b3": nc.dram_tensor("b3", [C, 1], f32, kind="ExternalInput").ap(),
        "oh": nc.dram_tensor("oh", [C, NL], f32, kind="ExternalInput").ap(),
        "out": nc.dram_tensor("out", [1, 1], f32, kind="ExternalOutput").ap(),
    }
    with tile.TileContext(nc) as tc:
        with ExitStack() as ctx:
            _emit(tc, ctx, aps, repeat, stage, hw_loop, prec)
    _strip_redundant_ldweights(nc)
    nc.compile()
    return nc


def _q8(x, s):
    return np.clip(x.astype(np.float32) * s, -240.0, 240.0).astype(E4M3)


def prep_inputs(X, Y, W1, b1, W2, b2, W3, b3, prec="fp8"):
    """Shard + retile (+ scale/quantize) the full inputs into per-core maps."""
    KD = D // 128
    KH = H // 128
    fp8 = prec == "fp8"

    if fp8:
        W1c = _q8(W1, SW)
        W2c = _q8(W2, SW)
        W3c = _q8(W3, SW)
        b1c = (b1.astype(np.float32) * SH)
        b2c = (b2.astype(np.float32) * SH)
    else:
        W1c, W2c, W3c = W1.astype(BF16), W2.astype(BF16), W3.astype(BF16)
        b1c, b2c = b1.astype(np.float32), b2.astype(np.float32)

    w1p = np.ascontiguousarray(W1c.reshape(KH, 128, KD, 128).transpose(0, 3, 2, 1))
    w2p = np.ascontiguousarray(W2c.reshape(KH, 128, KH, 128).transpose(0, 3, 2, 1))
    W3pad = np.zeros((CP, H), dtype=W3c.dtype)
    W3pad[:C] = W3c
    w3p = np.ascontiguousarray(W3pad.reshape(CP, KH, 128).transpose(2, 1, 0))
    b1p = np.ascontiguousarray(b1c.reshape(KH, 128).T)
    b2p = np.ascontiguousarray(b2c.reshape(KH, 128).T)
    b3p = np.ascontiguousarray(b3.astype(np.float32).reshape(C, 1))

    Xb = _q8(X, SX) if fp8 else X.astype(BF16)
    in_maps = []
    for c in range(N_CORES):
        Xc = Xb[c * NL:(c + 1) * NL]
        xtp = np.ascontiguousarray(Xc.reshape(NL, KD, 128).transpose(2, 1, 0))
        Yc = Y[c * NL:(c + 1) * NL]
        ohp = (np.arange(C, dtype=np.int64)[:, None] == Yc[None, :].astype(np.int64))
        ohp = np.ascontiguousarray(ohp.astype(np.float32))
        in_maps.append({
            "xt": xtp, "w1": w1p, "w2": w2p, "w3": w3p,
            "b1": b1p, "b2": b2p, "b3": b3p, "oh": ohp,
        })
    return in_maps


def log_prior(W1, b1, W2, b2, W3, b3):
    params = (W1, b1, W2, b2, W3, b3)
    d = sum(p.size for p in params)
    sq = sum(float(np.sum(p.astype(np.float64) ** 2)) for p in params)
    return -0.5 * (sq / PRIOR_VAR + d * math.log(2.0 * math.pi * PRIOR_VAR))


def _get_nc(repeat=1, hw_loop=False, prec="fp8"):
    key = (repeat, hw_loop, prec)
    if key not in _compiled:
        _compiled[key] = _build(repeat, hw_loop=hw_loop, prec=prec)
    return _compiled[key]


def run_device(in_maps, repeat=1, prec="fp8"):
    from concourse.bass_utils import run_bass_kernel_spmd

    nc = _get_nc(repeat, prec=prec)
    res = run_bass_kernel_spmd(nc, in_maps, list(range(N_CORES)))
    return [r["out"][0, 0] for r in res.results]


def kernel(X, Y, W1, b1, W2, b2, W3, b3):
    X = np.asarray(X)
    Y = np.asarray(Y)
    W1 = np.asarray(W1)
    b1 = np.asarray(b1)
    W2 = np.asarray(W2)
    b2 = np.asarray(b2)
    W3 = np.asarray(W3)
    b3 = np.asarray(b3)

    try:
        in_maps = prep_inputs(X, Y, W1, b1, W2, b2, W3, b3, prec="fp8")
        partials = run_device(in_maps, prec="fp8")
    except Exception:
        # Safety net: fp8 DoubleRow leans on newer walrus/ISA behavior; the
        # bf16 path is plain matmuls.
        in_maps = prep_inputs(X, Y, W1, b1, W2, b2, W3, b3, prec="bf16")
        partials = run_device(in_maps, prec="bf16")
    total = float(np.sum(np.asarray(partials, dtype=np.float64)))
    total += log_prior(W1, b1, W2, b2, W3, b3)
    return np.array(total, dtype=np.float32)
